# revision 1
# baseline (speedup 1.0000x reference)
"""MoE block kernel for Trainium2 (8 NeuronCores, data-parallel over batch).

Problem: B=8192, D=1024, H=256, E=16 experts, top-4 routing.
  logits = x @ route_w.T ; top4 softmax -> gates (B,E) (zeros elsewhere)
  out = sum_e gates[:,e] * relu(relu(x@W1e.T+b1e)@W2e.T+b2e)

Strategy: shard batch across 8 cores (1024 tokens each), replicate weights.
Each core computes all 16 experts densely (gates are 0 for unselected, so
dense-then-weight matches the reference exactly).

All matmuls run in fp16 (full PE rate; fp32/fp32r matmuls cannot carry sem
waits in this walrus build). The router needs near-fp32 logits so expert
selection never flips: x and route_w are split hi+lo in fp16 and the three
significant cross terms are accumulated in one PSUM group (err ~1e-7).
Expert matmuls use the fp16 hi part only (err ~1e-3, well within tolerance).

Layouts (host pre-transposed so contraction dims land on SBUF partitions):
  x_t_hi/lo (D, BL)  = fp16 split of x.T slice
  route_hi/lo (D, E) = fp16 split of route_w.T
  w1t (E, D, H), w2t (E, H, D) = fp16 weight transposes
mm1: hT[j,b] = sum_d w1t[d,j] * xT[d,b]  -> ACT relu(psum + b1[j]) -> fp16
mm2: y[b,do] = sum_j hT[j,b] * w2t[j,do] + ones[b]*b2[do] (K=1 bias matmul)
     ACT: t = relu(psum * gate[b])  (gate>=0 so relu(g*z)=g*relu(z)) -> fp16
     DVE: acc[b,do] += t   (fp16, 2x mode)
"""

import os
import sys

sys.path.insert(0, "/opt/trn_rl_repo")

import numpy as np

import concourse.bass as bass
import concourse.bacc as bacc
import concourse.mybir as mybir
import concourse.tile as tile
from concourse.bass_utils import run_bass_kernel_spmd

B, D, H, E = 8192, 1024, 256, 16
NCORES = 8
BL = B // NCORES  # 1024 tokens per core
P = 128
F32 = mybir.dt.float32
F16 = mybir.dt.float16
BBLK = 512

AX = mybir.AxisListType.X
AF = mybir.ActivationFunctionType
ALU = mybir.AluOpType


def build_nc():
    nc = bacc.Bacc("TRN2", target_bir_lowering=False, debug=False)
    x_hi = nc.declare_dram_parameter("x_hi", [D, BL], F16, isOutput=False)
    x_lo = nc.declare_dram_parameter("x_lo", [D, BL], F16, isOutput=False)
    r_hi = nc.declare_dram_parameter("r_hi", [D, E], F16, isOutput=False)
    r_lo = nc.declare_dram_parameter("r_lo", [D, E], F16, isOutput=False)
    w1t = nc.declare_dram_parameter("w1t", [E, D, H], F16, isOutput=False)
    w2t = nc.declare_dram_parameter("w2t", [E, H, D], F16, isOutput=False)
    b1 = nc.declare_dram_parameter("b1", [E, H], F32, isOutput=False)
    b2 = nc.declare_dram_parameter("b2", [E, D], F16, isOutput=False)
    out = nc.declare_dram_parameter("out", [BL, D], F16, isOutput=True)

    DT = D // P  # 8
    JT = H // P  # 2
    BT = BL // P  # 8
    NBB = BL // BBLK  # 2
    NSUB = BBLK // P  # 4
    NDO = D // BBLK  # 2

    with tile.TileContext(nc) as tc:
        with (
            tc.tile_pool(name="big", bufs=1) as big,
            tc.tile_pool(name="wts", bufs=2) as wts,
            tc.tile_pool(name="hbuf", bufs=3) as hbuf,
            tc.tile_pool(name="tbuf", bufs=6) as tbuf,
            tc.tile_pool(name="small", bufs=10) as small,
            tc.tile_pool(name="psh", bufs=2, space="PSUM") as psh_pool,
            tc.tile_pool(name="psy", bufs=4, space="PSUM") as psy_pool,
            tc.tile_pool(name="psr", bufs=2, space="PSUM") as psr_pool,
        ):
            # --- resident tensors ---
            xt_sb = big.tile([P, DT, BL], F16)  # 2MB
            nc.sync.dma_start(xt_sb, x_hi.rearrange("(o p) b -> p o b", p=P))
            xlo_sb = big.tile([P, DT, BL], F16)  # 2MB (router only)
            nc.sync.dma_start(xlo_sb, x_lo.rearrange("(o p) b -> p o b", p=P))
            rhi_sb = big.tile([P, DT, E], F16)
            nc.sync.dma_start(rhi_sb, r_hi.rearrange("(o p) e -> p o e", p=P))
            rlo_sb = big.tile([P, DT, E], F16)
            nc.sync.dma_start(rlo_sb, r_lo.rearrange("(o p) e -> p o e", p=P))
            ones_sb = big.tile([1, P], F16)
            nc.vector.memset(ones_sb, 1.0)
            acc = big.tile([P, BT, D], F16)  # 2MB output accumulator
            gates = big.tile([P, BT, E], F32)

            # --- router: logits = xhi@rhi + xhi@rlo + xlo@rhi (one group) ---
            for bt in range(BT):
                ps = psr_pool.tile([P, E], F32, tag="psr")
                groups = [(xt_sb, rhi_sb), (xt_sb, rlo_sb), (xlo_sb, rhi_sb)]
                n_mm = len(groups) * DT
                k = 0
                for xs, rs in groups:
                    for dt_i in range(DT):
                        nc.tensor.matmul(
                            ps,
                            lhsT=xs[:, dt_i, bt * P : (bt + 1) * P],
                            rhs=rs[:, dt_i, :],
                            start=(k == 0),
                            stop=(k == n_mm - 1),
                        )
                        k += 1
                logits = small.tile([P, E], F32, tag="logits")
                nc.vector.tensor_copy(logits, ps)
                m1 = small.tile([P, 1], F32, tag="m1")
                nc.vector.reduce_max(m1, logits, axis=AX)
                neg_m1 = small.tile([P, 1], F32, tag="negm1")
                nc.vector.tensor_scalar_mul(neg_m1, m1, -1.0)
                # knock out top-3, leaving mcur = 4th-largest logit
                tmp = small.tile([P, E], F32, tag="tmp")
                nc.vector.tensor_copy(tmp, logits)
                mcur = m1
                for it in range(3):
                    mask = small.tile([P, E], F32, tag=f"mask{it}")
                    nc.vector.tensor_scalar(mask, tmp, mcur, None, op0=ALU.is_ge)
                    nc.vector.scalar_tensor_tensor(
                        tmp, mask, -1e30, tmp, op0=ALU.mult, op1=ALU.add
                    )
                    mnext = small.tile([P, 1], F32, tag=f"mnext{it}")
                    nc.vector.reduce_max(mnext, tmp, axis=AX)
                    mcur = mnext
                maskt = small.tile([P, E], F32, tag="maskt")
                nc.vector.tensor_scalar(maskt, logits, mcur, None, op0=ALU.is_ge)
                expv = small.tile([P, E], F32, tag="expv")
                nc.scalar.activation(expv, logits, AF.Exp, bias=neg_m1, scale=1.0)
                expm = small.tile([P, E], F32, tag="expm")
                nc.vector.tensor_mul(expm, expv, maskt)
                ssum = small.tile([P, 1], F32, tag="ssum")
                nc.vector.reduce_sum(ssum, expm, axis=AX)
                rinv = small.tile([P, 1], F32, tag="rinv")
                nc.vector.reciprocal(rinv, ssum)
                nc.vector.tensor_scalar_mul(gates[:, bt, :], expm, rinv)

            # --- expert loop ---
            for e in range(E):
                w1_sb = wts.tile([P, DT, H], F16, tag="w1")
                nc.sync.dma_start(w1_sb, w1t[e].rearrange("(o p) h -> p o h", p=P))
                w2_sb = wts.tile([P, JT, D], F16, tag="w2")
                nc.sync.dma_start(w2_sb, w2t[e].rearrange("(o p) d -> p o d", p=P))
                b1_sb = wts.tile([P, JT], F32, tag="b1")
                nc.sync.dma_start(b1_sb, b1[e].rearrange("(o p) -> p o", p=P))
                b2row = wts.tile([1, D], F16, tag="b2")
                nc.sync.dma_start(b2row, b2[e][None, :])

                for bb in range(NBB):
                    hT = hbuf.tile([P, JT, BBLK], F16, tag="hT")
                    for jt in range(JT):
                        psh = psh_pool.tile([P, BBLK], F32, tag="psh")
                        for dt_i in range(DT):
                            nc.tensor.matmul(
                                psh,
                                lhsT=w1_sb[:, dt_i, jt * P : (jt + 1) * P],
                                rhs=xt_sb[:, dt_i, bb * BBLK : (bb + 1) * BBLK],
                                start=(dt_i == 0),
                                stop=(dt_i == DT - 1),
                            )
                        nc.scalar.activation(
                            hT[:, jt, :], psh, AF.Relu, bias=b1_sb[:, jt : jt + 1]
                        )
                    for bsub in range(NSUB):
                        bt = bb * NSUB + bsub
                        for dot in range(NDO):
                            psy = psy_pool.tile([P, BBLK], F32, tag="psy")
                            for jt in range(JT):
                                nc.tensor.matmul(
                                    psy,
                                    lhsT=hT[:, jt, bsub * P : (bsub + 1) * P],
                                    rhs=w2_sb[:, jt, dot * BBLK : (dot + 1) * BBLK],
                                    start=(jt == 0),
                                    stop=False,
                                )
                            nc.tensor.matmul(
                                psy,
                                lhsT=ones_sb,
                                rhs=b2row[:, dot * BBLK : (dot + 1) * BBLK],
                                start=False,
                                stop=True,
                            )
                            t = tbuf.tile([P, BBLK], F16, tag="t")
                            nc.scalar.activation(
                                t, psy, AF.Relu, scale=gates[:, bt, e : e + 1]
                            )
                            oslice = acc[:, bt, dot * BBLK : (dot + 1) * BBLK]
                            if e == 0:
                                nc.vector.tensor_copy(oslice, t)
                            else:
                                nc.vector.tensor_add(oslice, oslice, t)

            nc.sync.dma_start(out.rearrange("(o p) d -> p o d", p=P), acc)
    nc.compile()
    return nc


_NC_CACHE = None


def _get_nc():
    global _NC_CACHE
    if _NC_CACHE is None:
        _NC_CACHE = build_nc()
    return _NC_CACHE


def _split16(a):
    hi = a.astype(np.float16)
    lo = (a - hi.astype(np.float32)).astype(np.float16)
    return np.ascontiguousarray(hi), np.ascontiguousarray(lo)


def _prep_in_maps(x, route_w, w1, b1, w2, b2):
    x_t = np.asarray(x, dtype=np.float32).T  # (D, B)
    x_hi, x_lo = _split16(x_t)
    r_hi, r_lo = _split16(np.asarray(route_w, dtype=np.float32).T)
    w1t = np.ascontiguousarray(
        np.asarray(w1, dtype=np.float32).transpose(0, 2, 1).astype(np.float16)
    )
    w2t = np.ascontiguousarray(
        np.asarray(w2, dtype=np.float32).transpose(0, 2, 1).astype(np.float16)
    )
    b1 = np.ascontiguousarray(np.asarray(b1, dtype=np.float32))
    b2 = np.ascontiguousarray(np.asarray(b2, dtype=np.float32).astype(np.float16))
    in_maps = []
    for c in range(NCORES):
        sl = slice(c * BL, (c + 1) * BL)
        in_maps.append(
            {
                "x_hi": np.ascontiguousarray(x_hi[:, sl]),
                "x_lo": np.ascontiguousarray(x_lo[:, sl]),
                "r_hi": r_hi,
                "r_lo": r_lo,
                "w1t": w1t,
                "w2t": w2t,
                "b1": b1,
                "b2": b2,
            }
        )
    return in_maps


def run(x, route_w, w1, b1, w2, b2, trace=False, **trace_kw):
    nc = _get_nc()
    in_maps = _prep_in_maps(x, route_w, w1, b1, w2, b2)
    res = run_bass_kernel_spmd(
        nc, in_maps, list(range(NCORES)), trace=trace, **trace_kw
    )
    out = np.concatenate(
        [r["out"].astype(np.float32) for r in res.results], axis=0
    )
    return out, res


def kernel(x, route_w, w1, b1, w2, b2):
    out, _ = run(x, route_w, w1, b1, w2, b2, trace=False)
    return out

